# revision 4
# baseline (speedup 1.0000x reference)
"""Multi-head cross-attention TRN2 Bass kernel, sharded over 8 NeuronCores.

Problem (nn_MultiHeadCrossAttention): B=2, Sq=1024, Skv=4096 (text+image+
audio+video), hidden=1024, heads=16, head_dim=64, out=4096.

Sharding: core c = 4*b + hg handles batch b and head-group hg (4 heads).
Per core (all matmuls in float32r: ~bf16 speed, ~1e-4 accuracy):
  QT proj:  QT[d,sq]   = Wq_g  @ ff[b].T      (ffT streamed, contraction 4096)
  KT proj:  KT[d,kv]   = Wk_g  @ kv[b].T      (kvT streamed, contraction 1024)
  V  proj:  V[kv,d]    = kv[b] @ Wv_g.T       (natural layout, 65th col = ones)
  scores^T: S[kv,sq]   = K^T q  (row-tiled K=64 matmul pairs)
  softmax:  P = exp(S/8) (no max-subtract: |scores| <~ 3 for this data)
  PV:       att[d,sq] += V_ext^T @ P  (M=65: row 64 accumulates denominator)
  norm:     att = att * recip(den) (K=1 broadcast matmul expands recip row)
  out-proj: outT[j,sq] = Wo[:, fslice].T.T @ attT  -> partial over f-slice
Host sums the 4 per-batch partials and adds bo.
"""

import numpy as np

import bass_rust
import concourse.bass as bass
import concourse.mybir as mybir
import concourse.tile as tile
from concourse.bass_utils import run_bass_kernel_spmd
from concourse.vector_clock import ScopedClock

# ---------------------------------------------------------------------------
# Workarounds for walrus per-instruction sync-wait caps (this walrus build
# rejects instructions carrying more waits than the ISA slot count; Tile's
# sem assignment can attach more). Split excess waits onto single-wait nops.
# ---------------------------------------------------------------------------
import re as _re

_VC_RE = _re.compile(r"VectorClock\(\[([0-9, ]*)\]\)")


def _vc_values(vc):
    m = _VC_RE.match(repr(vc))
    assert m, repr(vc)
    s = m.group(1).strip()
    return [int(x) for x in s.split(",")] if s else []


def _split_excess_waits(tc, ordered_instructions_by_block, max_waits=1):
    nc = tc.nc
    for _bb, insts in ordered_instructions_by_block.items():
        out = []
        for inst in insts:
            si = inst.sync_info
            waits = list(si.on_wait) if si and si.on_wait else []
            if len(waits) > max_waits:
                keep = waits[:max_waits]
                for w in waits[max_waits:]:
                    nop = mybir.InstNoOp(
                        name=nc.get_next_instruction_name(), ins=[], outs=[]
                    )
                    nop.engine = inst.engine
                    nop.sync_info = bass_rust.SyncInfo(on_wait=[w], on_update=[])
                    nc.register_instruction(nop)
                    out.append(nop)
                inst.sync_info = bass_rust.SyncInfo(
                    on_wait=keep, on_update=list(si.on_update or [])
                )
            out.append(inst)
        insts[:] = out


_orig_lower = tile.TileContext._lower_ordered_insts


def _lower_with_split(self, postordered_blocks):
    _split_excess_waits(self, postordered_blocks)
    return _orig_lower(self, postordered_blocks)


def _drain_and_barrier_split(self, tick_clock, wait_clock):
    vals = _vc_values(tick_clock.global_clock)
    for proc_idx, tick in enumerate(vals):
        if tick <= 0:
            continue
        single = [0] * len(vals)
        single[proc_idx] = tick
        nop_inst = self.nc.sync.nop(nofuse=True, hint=f"drain_wait_p{proc_idx}")
        wait_clock.add_sem_waits(
            nop_inst.ins, ScopedClock({None: bass_rust.VectorClock(single)})
        )
    self.nc.sync.drain()
    self.nc.all_engine_barrier()
    assert self.sems is not None
    popped = self.nc._tile_sem_poison_stack.pop()
    assert popped is self._sem_poison
    self.nc.clear_and_free_semaphores(list(self.sems.allocated().values()))
    self.nc.all_engine_barrier()


tile.TileContext._lower_ordered_insts = _lower_with_split
tile.TileContext._drain_and_barrier = _drain_and_barrier_split

# ---------------------------------------------------------------------------
# Problem constants (hardcoded per contract)
# ---------------------------------------------------------------------------
B = 2
SQ = 1024
SKV = 4096
HID = 1024
HEADS = 16
DH = 64
DOUT = 4096
NCORES = 8
HG = 4  # head-groups (cores per batch)
GHEADS = HEADS // HG  # heads per group = 4
GF = GHEADS * DH  # feature slice per group = 256
NPAIR = GHEADS // 2  # head pairs per group = 2

F32 = mybir.dt.float32
F32R = mybir.dt.float32r
Exp = mybir.ActivationFunctionType.Exp
MUL = mybir.AluOpType.mult
ADD = mybir.AluOpType.add

NKVT = SKV // 128  # 32 kv tiles
NKVB = 8  # kv blocks (512 wide)
NFT_Q = 4096 // 128  # 32 contraction tiles for Q proj
NFT_KV = HID // 128  # 8 contraction tiles for K/V proj
NSQH = SQ // 512  # 2 sq halves
NJT = DOUT // 128  # 32 output row tiles

_NC_CACHE = {}


def build():
    if "nc" in _NC_CACHE:
        return _NC_CACHE["nc"]
    nc = bass.Bass()

    fft = nc.declare_dram_parameter("fft", [4096, SQ], F32, isOutput=False)
    kvt = nc.declare_dram_parameter("kvt", [HID, SKV], F32, isOutput=False)
    wqt = nc.declare_dram_parameter("wqt", [4096, GF], F32, isOutput=False)
    wkt = nc.declare_dram_parameter("wkt", [HID, GF], F32, isOutput=False)
    wvt = nc.declare_dram_parameter("wvt", [HID, GF], F32, isOutput=False)
    wot = nc.declare_dram_parameter("wot", [GF, DOUT], F32, isOutput=False)
    bq = nc.declare_dram_parameter("bq", [128, NPAIR], F32, isOutput=False)
    bk = nc.declare_dram_parameter("bk", [128, NPAIR], F32, isOutput=False)
    bv = nc.declare_dram_parameter("bv", [128, NPAIR], F32, isOutput=False)
    outp = nc.declare_dram_parameter("outp", [DOUT, SQ], F32, isOutput=True)

    with tile.TileContext(nc) as tc:
        with (
            tc.tile_pool(name="hold", bufs=1) as hold,
            tc.tile_pool(name="misc", bufs=1) as misc,
        ):
            # ---- long-lived tiles ----
            wkt_r = hold.tile([128, NFT_KV, NPAIR, 128], F32R, tag="wkt")
            nc.gpsimd.dma_start(
                out=wkt_r[:],
                in_=wkt.rearrange("(ft p) (pr d) -> p ft pr d", p=128, pr=NPAIR),
            )
            wvt_r = hold.tile([128, NFT_KV, GF], F32R, tag="wvt")
            nc.gpsimd.dma_start(
                out=wvt_r[:], in_=wvt.rearrange("(ft p) d -> p ft d", p=128)
            )
            wot_r = hold.tile([128, NPAIR, DOUT], F32R, tag="wot")
            nc.gpsimd.dma_start(
                out=wot_r[:], in_=wot.rearrange("(pr p) j -> p pr j", p=128)
            )
            bq_t = misc.tile([128, NPAIR], F32, tag="bq")
            nc.sync.dma_start(out=bq_t[:], in_=bq[:])
            bk_t = misc.tile([128, NPAIR], F32, tag="bk")
            nc.sync.dma_start(out=bk_t[:], in_=bk[:])
            bv_t = misc.tile([128, NPAIR], F32, tag="bv")
            nc.sync.dma_start(out=bv_t[:], in_=bv[:])

            ones_f = misc.tile([128, GHEADS], F32, tag="ones_f")
            nc.vector.memset(ones_f[:], 1.0)
            ones_row = misc.tile([1, DH], F32R, tag="ones_row")
            nc.vector.tensor_copy(ones_row[:], ones_f[0:1, 0:1].broadcast_to([1, DH]))

            qt_r = hold.tile([128, NPAIR, SQ], F32R, tag="qt")
            kt_r = hold.tile([128, NPAIR, SKV], F32R, tag="kt")
            v_r = hold.tile([128, NKVT, GHEADS, DH + 1], F32R, tag="v")
            att_r = hold.tile([128, NPAIR, SQ], F32R, tag="att")

            # ================= Phase A: QT projection =================
            with (
                tc.tile_pool(name="ffts", bufs=3) as ffts,
                tc.tile_pool(name="wqs", bufs=3) as wqs,
                tc.tile_pool(name="psA", bufs=4, space="PSUM") as psA,
            ):
                qt_ps = [
                    psA.tile([128, 512], F32, tag="psA", name=f"qt_ps{i}")
                    for i in range(4)
                ]  # (pair, sqh)
                for kt in range(NFT_Q):
                    fft_t = ffts.tile([128, SQ], F32R, tag="fft")
                    nc.gpsimd.dma_start(
                        out=fft_t[:], in_=fft[128 * kt : 128 * (kt + 1), :]
                    )
                    wq_t = wqs.tile([128, NPAIR, 128], F32R, tag="wq")
                    nc.gpsimd.dma_start(
                        out=wq_t[:],
                        in_=wqt[128 * kt : 128 * (kt + 1), :].rearrange(
                            "p (pr d) -> p pr d", pr=NPAIR
                        ),
                    )
                    for pr in range(NPAIR):
                        for sh in range(NSQH):
                            nc.tensor.matmul(
                                qt_ps[pr * NSQH + sh][:],
                                wq_t[:, pr, :],
                                fft_t[:, 512 * sh : 512 * (sh + 1)],
                                start=(kt == 0),
                                stop=(kt == NFT_Q - 1),
                            )
                for pr in range(NPAIR):
                    for sh in range(NSQH):
                        nc.vector.tensor_scalar(
                            qt_r[:, pr, 512 * sh : 512 * (sh + 1)],
                            qt_ps[pr * NSQH + sh][:],
                            bq_t[:, pr : pr + 1],
                            None,
                            ADD,
                        )

            # ============ Phase B: KT + V projections (kv blocks) ============
            with (
                tc.tile_pool(name="kvs", bufs=2) as kvs,
                tc.tile_pool(name="psB", bufs=4, space="PSUM") as psB,
            ):
                for kb in range(NKVB):
                    kv_t = kvs.tile([128, NFT_KV, 512], F32R, tag="kv")
                    nc.gpsimd.dma_start(
                        out=kv_t[:],
                        in_=kvt[:, 512 * kb : 512 * (kb + 1)].rearrange(
                            "(ft p) n -> p ft n", p=128
                        ),
                    )
                    for pr in range(NPAIR):
                        kt_ps = psB.tile([128, 512], F32, tag="psB")
                        for ft in range(NFT_KV):
                            nc.tensor.matmul(
                                kt_ps[:],
                                wkt_r[:, ft, pr, :],
                                kv_t[:, ft, :],
                                start=(ft == 0),
                                stop=(ft == NFT_KV - 1),
                            )
                        nc.vector.tensor_scalar(
                            kt_r[:, pr, 512 * kb : 512 * (kb + 1)],
                            kt_ps[:],
                            bk_t[:, pr : pr + 1],
                            None,
                            ADD,
                        )
                    for kl in range(4):
                        kvt_i = kb * 4 + kl
                        v_ps = psB.tile([128, GF], F32, tag="psB")
                        for ft in range(NFT_KV):
                            nc.tensor.matmul(
                                v_ps[:],
                                kv_t[:, ft, 128 * kl : 128 * (kl + 1)],
                                wvt_r[:, ft, :],
                                start=(ft == 0),
                                stop=(ft == NFT_KV - 1),
                            )
                        nc.vector.tensor_copy(
                            v_r[:, kvt_i, :, 0:DH],
                            v_ps.rearrange("p (h d) -> p h d", h=GHEADS),
                        )
                        nc.vector.tensor_copy(
                            v_r[:, kvt_i, :, DH : DH + 1], ones_f[:, :]
                        )

            # ================= Phase C: attention =================
            with (
                tc.tile_pool(name="pp", bufs=3) as pp,
                tc.tile_pool(name="nrm", bufs=2) as nrm,
                tc.tile_pool(name="psS", bufs=4, space="PSUM") as psS,
                tc.tile_pool(name="psAtt", bufs=2, space="PSUM") as psAtt,
                tc.tile_pool(name="psRb", bufs=2, space="PSUM") as psRb,
            ):
                for pr in range(NPAIR):
                    for sh in range(NSQH):
                        sq_sl = slice(512 * sh, 512 * (sh + 1))
                        att0 = psAtt.tile([DH + 1, 512], F32, tag="psAtt")
                        att1 = psAtt.tile([DH + 1, 512], F32, tag="psAtt")
                        for kv in range(NKVT):
                            s0 = psS.tile([128, 512], F32, tag="psS")
                            s1 = psS.tile([128, 512], F32, tag="psS")
                            kv_sl = slice(128 * kv, 128 * (kv + 1))
                            nc.tensor.matmul(
                                s0[:],
                                kt_r[0:DH, pr, kv_sl],
                                qt_r[0:DH, pr, sq_sl],
                                start=True,
                                stop=True,
                            )
                            nc.tensor.matmul(
                                s1[:],
                                kt_r[DH:128, pr, kv_sl],
                                qt_r[DH:128, pr, sq_sl],
                                start=True,
                                stop=True,
                            )
                            p0 = pp.tile([128, 512], F32R, tag="p0")
                            p1 = pp.tile([128, 512], F32R, tag="p1")
                            nc.scalar.activation(p0[:], s0[:], Exp, scale=0.125)
                            nc.scalar.activation(p1[:], s1[:], Exp, scale=0.125)
                            first, last = kv == 0, kv == NKVT - 1
                            nc.tensor.matmul(
                                att0[:],
                                v_r[:, kv, 2 * pr, :],
                                p0[:],
                                start=first,
                                stop=last,
                            )
                            nc.tensor.matmul(
                                att1[:],
                                v_r[:, kv, 2 * pr + 1, :],
                                p1[:],
                                start=first,
                                stop=last,
                            )
                        # normalize pair
                        rec0 = nrm.tile([1, 512], F32R, tag="rec0")
                        rec1 = nrm.tile([1, 512], F32R, tag="rec1")
                        with nc.allow_low_precision(reason="softmax recip"):
                            nc.vector.reciprocal(rec0[:], att0[DH : DH + 1, :])
                            nc.vector.reciprocal(rec1[:], att1[DH : DH + 1, :])
                        rb0 = psRb.tile([DH, 512], F32, tag="psRb")
                        rb1 = psRb.tile([DH, 512], F32, tag="psRb")
                        nc.tensor.matmul(
                            rb0[:], ones_row[0:1, :], rec0[0:1, :],
                            start=True, stop=True,
                        )
                        nc.tensor.matmul(
                            rb1[:], ones_row[0:1, :], rec1[0:1, :],
                            start=True, stop=True,
                        )
                        rb_sb = nrm.tile([128, 512], F32, tag="rbsb")
                        nc.vector.tensor_copy(rb_sb[0:DH, :], rb0[:])
                        nc.vector.tensor_copy(rb_sb[DH:128, :], rb1[:])
                        mulx = nrm.tile([128, 512], F32, tag="mulx")
                        nc.vector.tensor_tensor(
                            mulx[0:DH, :], att0[0:DH, :], rb_sb[0:DH, :], MUL
                        )
                        nc.vector.tensor_tensor(
                            mulx[DH:128, :], att1[0:DH, :], rb_sb[DH:128, :], MUL
                        )
                        nc.vector.tensor_scalar(
                            att_r[:, pr, sq_sl],
                            mulx[:],
                            bv_t[:, pr : pr + 1],
                            None,
                            ADD,
                        )

            # ================= Phase D: out projection =================
            with (
                tc.tile_pool(name="osb", bufs=3) as osb,
                tc.tile_pool(name="psD", bufs=4, space="PSUM") as psD,
            ):
                for jt in range(NJT):
                    o_ps = [psD.tile([128, 512], F32, tag="psD", name=f"o_ps{jt}_{i}") for i in range(NSQH)]
                    j_sl = slice(128 * jt, 128 * (jt + 1))
                    for pr in range(NPAIR):
                        for sh in range(NSQH):
                            nc.tensor.matmul(
                                o_ps[sh][:],
                                wot_r[:, pr, j_sl],
                                att_r[:, pr, 512 * sh : 512 * (sh + 1)],
                                start=(pr == 0),
                                stop=(pr == NPAIR - 1),
                            )
                    o_sb = osb.tile([128, SQ], F32, tag="osb")
                    for sh in range(NSQH):
                        nc.vector.tensor_copy(
                            o_sb[:, 512 * sh : 512 * (sh + 1)], o_ps[sh][:]
                        )
                    nc.sync.dma_start(out=outp[j_sl, :], in_=o_sb[:])

    _NC_CACHE["nc"] = nc
    return nc


def _make_in_maps(inputs):
    ff = np.asarray(inputs["fused_features"], dtype=np.float32)
    kv_in = np.concatenate(
        [
            np.asarray(inputs["text"], dtype=np.float32),
            np.asarray(inputs["image"], dtype=np.float32),
            np.asarray(inputs["audio"], dtype=np.float32),
            np.asarray(inputs["video"], dtype=np.float32),
        ],
        axis=1,
    )
    Wq = np.asarray(inputs["Wq"], dtype=np.float32)
    Wk = np.asarray(inputs["Wk"], dtype=np.float32)
    Wv = np.asarray(inputs["Wv"], dtype=np.float32)
    Wo = np.asarray(inputs["Wo"], dtype=np.float32)
    bq = np.asarray(inputs["bq"], dtype=np.float32)
    bk = np.asarray(inputs["bk"], dtype=np.float32)
    bv = np.asarray(inputs["bv"], dtype=np.float32)

    ffT = [np.ascontiguousarray(ff[b].T) for b in range(B)]
    kvT = [np.ascontiguousarray(kv_in[b].T) for b in range(B)]
    WqT = np.ascontiguousarray(Wq.T)  # [4096, 1024]
    WkT = np.ascontiguousarray(Wk.T)  # [1024, 1024]
    WvT = np.ascontiguousarray(Wv.T)
    WoT = np.ascontiguousarray(Wo.T)  # [1024, 4096]

    in_maps = []
    for c in range(NCORES):
        b, hg = divmod(c, HG)
        fs = slice(GF * hg, GF * (hg + 1))
        in_maps.append(
            {
                "fft": ffT[b],
                "kvt": kvT[b],
                "wqt": np.ascontiguousarray(WqT[:, fs]),
                "wkt": np.ascontiguousarray(WkT[:, fs]),
                "wvt": np.ascontiguousarray(WvT[:, fs]),
                "wot": np.ascontiguousarray(WoT[fs, :]),
                "bq": np.ascontiguousarray(bq[fs].reshape(NPAIR, 128).T),
                "bk": np.ascontiguousarray(bk[fs].reshape(NPAIR, 128).T),
                "bv": np.ascontiguousarray(bv[fs].reshape(NPAIR, 128).T),
            }
        )
    return in_maps


def _assemble(results, bo):
    out = np.zeros((B, SQ, DOUT), dtype=np.float32)
    for c in range(NCORES):
        b = c // HG
        out[b] += results[c]["outp"].T
    out += np.asarray(bo, dtype=np.float32)
    return out


def run_spmd(inputs, trace=False):
    nc = build()
    in_maps = _make_in_maps(inputs)
    r = run_bass_kernel_spmd(nc, in_maps, list(range(NCORES)), trace=trace)
    return _assemble(r.results, inputs["bo"]), r


def kernel(**inputs) -> np.ndarray:
    out, _ = run_spmd(inputs, trace=False)
    return out


# revision 5
# speedup vs baseline: 1.4450x; 1.4450x over previous
"""Multi-head cross-attention TRN2 Bass kernel, sharded over 8 NeuronCores.

Problem (nn_MultiHeadCrossAttention): B=2, Sq=1024, Skv=4096 (text+image+
audio+video), hidden=1024, heads=16, head_dim=64, out=4096.

Sharding: core c = 4*b + hg handles batch b and head-group hg (4 heads).
Per core (all matmuls in float32r: ~bf16 speed, ~1e-4 accuracy):
  QT proj:  QT[d,sq]   = Wq_g  @ ff[b].T      (ffT streamed, contraction 4096)
  KT proj:  KT[d,kv]   = Wk_g  @ kv[b].T      (kvT streamed, contraction 1024)
  V  proj:  V[kv,d]    = kv[b] @ Wv_g.T       (natural layout, 65th col = ones)
  scores^T: S[kv,sq]   = K^T q  (row-tiled K=64 matmul pairs)
  softmax:  P = exp(S/8) (no max-subtract: |scores| <~ 3 for this data)
  PV:       att[d,sq] += V_ext^T @ P  (M=65: row 64 accumulates denominator)
  norm:     att = att * recip(den) (K=1 broadcast matmul expands recip row)
  out-proj: outT[j,sq] = Wo[:, fslice].T.T @ attT  -> partial over f-slice
Host sums the 4 per-batch partials and adds bo.
"""

import numpy as np

import bass_rust
import concourse.bass as bass
import concourse.mybir as mybir
import concourse.tile as tile
from concourse.bass_utils import run_bass_kernel_spmd
from concourse.vector_clock import ScopedClock

# ---------------------------------------------------------------------------
# Workarounds for walrus per-instruction sync-wait caps (this walrus build
# rejects instructions carrying more waits than the ISA slot count; Tile's
# sem assignment can attach more). Split excess waits onto single-wait nops.
# ---------------------------------------------------------------------------
import re as _re

_VC_RE = _re.compile(r"VectorClock\(\[([0-9, ]*)\]\)")


def _vc_values(vc):
    m = _VC_RE.match(repr(vc))
    assert m, repr(vc)
    s = m.group(1).strip()
    return [int(x) for x in s.split(",")] if s else []


def _split_excess_waits(tc, ordered_instructions_by_block, max_waits=1):
    nc = tc.nc
    for _bb, insts in ordered_instructions_by_block.items():
        out = []
        for inst in insts:
            si = inst.sync_info
            waits = list(si.on_wait) if si and si.on_wait else []
            if len(waits) > max_waits:
                keep = waits[:max_waits]
                for w in waits[max_waits:]:
                    nop = mybir.InstNoOp(
                        name=nc.get_next_instruction_name(), ins=[], outs=[]
                    )
                    nop.engine = inst.engine
                    nop.sync_info = bass_rust.SyncInfo(on_wait=[w], on_update=[])
                    nc.register_instruction(nop)
                    out.append(nop)
                inst.sync_info = bass_rust.SyncInfo(
                    on_wait=keep, on_update=list(si.on_update or [])
                )
            out.append(inst)
        insts[:] = out


_orig_lower = tile.TileContext._lower_ordered_insts


def _lower_with_split(self, postordered_blocks):
    _split_excess_waits(self, postordered_blocks)
    return _orig_lower(self, postordered_blocks)


def _drain_and_barrier_split(self, tick_clock, wait_clock):
    vals = _vc_values(tick_clock.global_clock)
    for proc_idx, tick in enumerate(vals):
        if tick <= 0:
            continue
        single = [0] * len(vals)
        single[proc_idx] = tick
        nop_inst = self.nc.sync.nop(nofuse=True, hint=f"drain_wait_p{proc_idx}")
        wait_clock.add_sem_waits(
            nop_inst.ins, ScopedClock({None: bass_rust.VectorClock(single)})
        )
    self.nc.sync.drain()
    self.nc.all_engine_barrier()
    assert self.sems is not None
    popped = self.nc._tile_sem_poison_stack.pop()
    assert popped is self._sem_poison
    self.nc.clear_and_free_semaphores(list(self.sems.allocated().values()))
    self.nc.all_engine_barrier()


tile.TileContext._lower_ordered_insts = _lower_with_split
tile.TileContext._drain_and_barrier = _drain_and_barrier_split

# ---------------------------------------------------------------------------
# Problem constants (hardcoded per contract)
# ---------------------------------------------------------------------------
B = 2
SQ = 1024
SKV = 4096
HID = 1024
HEADS = 16
DH = 64
DOUT = 4096
NCORES = 8
HG = 4  # head-groups (cores per batch)
GHEADS = HEADS // HG  # heads per group = 4
GF = GHEADS * DH  # feature slice per group = 256
NPAIR = GHEADS // 2  # head pairs per group = 2

F32 = mybir.dt.float32
F32R = mybir.dt.float32r
BF16 = mybir.dt.bfloat16
DT_MM = BF16  # matmul operand dtype: BF16 (fast ldweights) or F32R (accuracy)
NP_MM = "bfloat16"  # host-side dtype name for DT_MM inputs
Exp = mybir.ActivationFunctionType.Exp
MUL = mybir.AluOpType.mult
ADD = mybir.AluOpType.add

NKVT = SKV // 128  # 32 kv tiles
NKVB = 8  # kv blocks (512 wide)
NFT_Q = 4096 // 128  # 32 contraction tiles for Q proj
NFT_KV = HID // 128  # 8 contraction tiles for K/V proj
NSQH = SQ // 512  # 2 sq halves
NJT = DOUT // 128  # 32 output row tiles

_NC_CACHE = {}


def build():
    if "nc" in _NC_CACHE:
        return _NC_CACHE["nc"]
    nc = bass.Bass()

    fft = nc.declare_dram_parameter("fft", [4096, SQ], DT_MM, isOutput=False)
    kvt = nc.declare_dram_parameter("kvt", [HID, SKV], DT_MM, isOutput=False)
    wqt = nc.declare_dram_parameter("wqt", [4096, GF], DT_MM, isOutput=False)
    wkt = nc.declare_dram_parameter("wkt", [HID, GF], DT_MM, isOutput=False)
    wvt = nc.declare_dram_parameter("wvt", [HID, GF], DT_MM, isOutput=False)
    wot = nc.declare_dram_parameter("wot", [GF, DOUT], DT_MM, isOutput=False)
    bq = nc.declare_dram_parameter("bq", [128, NPAIR], F32, isOutput=False)
    bk = nc.declare_dram_parameter("bk", [128, NPAIR], F32, isOutput=False)
    bv = nc.declare_dram_parameter("bv", [128, NPAIR], F32, isOutput=False)
    outp = nc.declare_dram_parameter("outp", [DOUT, SQ], F32, isOutput=True)

    with tile.TileContext(nc) as tc:
        with (
            tc.tile_pool(name="hold", bufs=1) as hold,
            tc.tile_pool(name="misc", bufs=1) as misc,
        ):
            # ---- long-lived tiles ----
            wkt_r = hold.tile([128, NFT_KV, NPAIR, 128], DT_MM, tag="wkt")
            nc.sync.dma_start(
                out=wkt_r[:],
                in_=wkt.rearrange("(ft p) (pr d) -> p ft pr d", p=128, pr=NPAIR),
            )
            wvt_r = hold.tile([128, NFT_KV, GF], DT_MM, tag="wvt")
            nc.sync.dma_start(
                out=wvt_r[:], in_=wvt.rearrange("(ft p) d -> p ft d", p=128)
            )
            wot_r = hold.tile([128, NPAIR, DOUT], DT_MM, tag="wot")
            nc.sync.dma_start(
                out=wot_r[:], in_=wot.rearrange("(pr p) j -> p pr j", p=128)
            )
            bq_t = misc.tile([128, NPAIR], F32, tag="bq")
            nc.sync.dma_start(out=bq_t[:], in_=bq[:])
            bk_t = misc.tile([128, NPAIR], F32, tag="bk")
            nc.sync.dma_start(out=bk_t[:], in_=bk[:])
            bv_t = misc.tile([128, NPAIR], F32, tag="bv")
            nc.sync.dma_start(out=bv_t[:], in_=bv[:])

            ones_f = misc.tile([128, GHEADS], F32, tag="ones_f")
            nc.vector.memset(ones_f[:], 1.0)
            ones_row = misc.tile([1, DH], DT_MM, tag="ones_row")
            nc.vector.tensor_copy(ones_row[:], ones_f[0:1, 0:1].broadcast_to([1, DH]))

            qt_r = hold.tile([128, NPAIR, SQ], DT_MM, tag="qt")
            kt_r = hold.tile([128, NPAIR, SKV], DT_MM, tag="kt")
            v_r = hold.tile([128, NKVT, GHEADS, DH + 1], DT_MM, tag="v")
            att_r = hold.tile([128, NPAIR, SQ], DT_MM, tag="att")

            # ================= Phase A: QT projection =================
            with (
                tc.tile_pool(name="ffts", bufs=3) as ffts,
                tc.tile_pool(name="wqs", bufs=3) as wqs,
                tc.tile_pool(name="psA", bufs=4, space="PSUM") as psA,
            ):
                qt_ps = [
                    psA.tile([128, 512], F32, tag="psA", name=f"qt_ps{i}")
                    for i in range(4)
                ]  # (pair, sqh)
                for kt in range(NFT_Q):
                    fft_t = ffts.tile([128, SQ], DT_MM, tag="fft")
                    nc.sync.dma_start(
                        out=fft_t[:], in_=fft[128 * kt : 128 * (kt + 1), :]
                    )
                    wq_t = wqs.tile([128, NPAIR, 128], DT_MM, tag="wq")
                    nc.sync.dma_start(
                        out=wq_t[:],
                        in_=wqt[128 * kt : 128 * (kt + 1), :].rearrange(
                            "p (pr d) -> p pr d", pr=NPAIR
                        ),
                    )
                    for pr in range(NPAIR):
                        for sh in range(NSQH):
                            nc.tensor.matmul(
                                qt_ps[pr * NSQH + sh][:],
                                wq_t[:, pr, :],
                                fft_t[:, 512 * sh : 512 * (sh + 1)],
                                start=(kt == 0),
                                stop=(kt == NFT_Q - 1),
                            )
                for pr in range(NPAIR):
                    for sh in range(NSQH):
                        nc.vector.tensor_scalar(
                            qt_r[:, pr, 512 * sh : 512 * (sh + 1)],
                            qt_ps[pr * NSQH + sh][:],
                            bq_t[:, pr : pr + 1],
                            None,
                            ADD,
                        )

            # ============ Phase B: KT + V projections (kv blocks) ============
            with (
                tc.tile_pool(name="kvs", bufs=2) as kvs,
                tc.tile_pool(name="psB", bufs=4, space="PSUM") as psB,
            ):
                for kb in range(NKVB):
                    kv_t = kvs.tile([128, NFT_KV, 512], DT_MM, tag="kv")
                    nc.sync.dma_start(
                        out=kv_t[:],
                        in_=kvt[:, 512 * kb : 512 * (kb + 1)].rearrange(
                            "(ft p) n -> p ft n", p=128
                        ),
                    )
                    for pr in range(NPAIR):
                        kt_ps = psB.tile([128, 512], F32, tag="psB")
                        for ft in range(NFT_KV):
                            nc.tensor.matmul(
                                kt_ps[:],
                                wkt_r[:, ft, pr, :],
                                kv_t[:, ft, :],
                                start=(ft == 0),
                                stop=(ft == NFT_KV - 1),
                            )
                        nc.vector.tensor_scalar(
                            kt_r[:, pr, 512 * kb : 512 * (kb + 1)],
                            kt_ps[:],
                            bk_t[:, pr : pr + 1],
                            None,
                            ADD,
                        )
                    for kl in range(4):
                        kvt_i = kb * 4 + kl
                        v_ps = psB.tile([128, GF], F32, tag="psB")
                        for ft in range(NFT_KV):
                            nc.tensor.matmul(
                                v_ps[:],
                                kv_t[:, ft, 128 * kl : 128 * (kl + 1)],
                                wvt_r[:, ft, :],
                                start=(ft == 0),
                                stop=(ft == NFT_KV - 1),
                            )
                        nc.vector.tensor_copy(
                            v_r[:, kvt_i, :, 0:DH],
                            v_ps.rearrange("p (h d) -> p h d", h=GHEADS),
                        )
                        nc.vector.tensor_copy(
                            v_r[:, kvt_i, :, DH : DH + 1], ones_f[:, :]
                        )

            # ================= Phase C: attention =================
            with (
                tc.tile_pool(name="pp", bufs=3) as pp,
                tc.tile_pool(name="nrm", bufs=2) as nrm,
                tc.tile_pool(name="psS", bufs=4, space="PSUM") as psS,
                tc.tile_pool(name="psAtt", bufs=2, space="PSUM") as psAtt,
                tc.tile_pool(name="psRb", bufs=2, space="PSUM") as psRb,
            ):
                for pr in range(NPAIR):
                    for sh in range(NSQH):
                        sq_sl = slice(512 * sh, 512 * (sh + 1))
                        att0 = psAtt.tile([DH + 1, 512], F32, tag="psAtt")
                        att1 = psAtt.tile([DH + 1, 512], F32, tag="psAtt")
                        for kv in range(NKVT):
                            s0 = psS.tile([128, 512], F32, tag="psS")
                            s1 = psS.tile([128, 512], F32, tag="psS")
                            kv_sl = slice(128 * kv, 128 * (kv + 1))
                            nc.tensor.matmul(
                                s0[:],
                                kt_r[0:DH, pr, kv_sl],
                                qt_r[0:DH, pr, sq_sl],
                                start=True,
                                stop=True,
                            )
                            nc.tensor.matmul(
                                s1[:],
                                kt_r[DH:128, pr, kv_sl],
                                qt_r[DH:128, pr, sq_sl],
                                start=True,
                                stop=True,
                            )
                            p0 = pp.tile([128, 512], DT_MM, tag="p0")
                            p1 = pp.tile([128, 512], DT_MM, tag="p1")
                            nc.scalar.activation(p0[:], s0[:], Exp, scale=0.125)
                            nc.scalar.activation(p1[:], s1[:], Exp, scale=0.125)
                            first, last = kv == 0, kv == NKVT - 1
                            nc.tensor.matmul(
                                att0[:],
                                v_r[:, kv, 2 * pr, :],
                                p0[:],
                                start=first,
                                stop=last,
                            )
                            nc.tensor.matmul(
                                att1[:],
                                v_r[:, kv, 2 * pr + 1, :],
                                p1[:],
                                start=first,
                                stop=last,
                            )
                        # normalize pair
                        rec0 = nrm.tile([1, 512], DT_MM, tag="rec0")
                        rec1 = nrm.tile([1, 512], DT_MM, tag="rec1")
                        with nc.allow_low_precision(reason="softmax recip"):
                            nc.vector.reciprocal(rec0[:], att0[DH : DH + 1, :])
                            nc.vector.reciprocal(rec1[:], att1[DH : DH + 1, :])
                        rb0 = psRb.tile([DH, 512], F32, tag="psRb")
                        rb1 = psRb.tile([DH, 512], F32, tag="psRb")
                        nc.tensor.matmul(
                            rb0[:], ones_row[0:1, :], rec0[0:1, :],
                            start=True, stop=True,
                        )
                        nc.tensor.matmul(
                            rb1[:], ones_row[0:1, :], rec1[0:1, :],
                            start=True, stop=True,
                        )
                        rb_sb = nrm.tile([128, 512], F32, tag="rbsb")
                        nc.vector.tensor_copy(rb_sb[0:DH, :], rb0[:])
                        nc.vector.tensor_copy(rb_sb[DH:128, :], rb1[:])
                        mulx = nrm.tile([128, 512], F32, tag="mulx")
                        nc.vector.tensor_tensor(
                            mulx[0:DH, :], att0[0:DH, :], rb_sb[0:DH, :], MUL
                        )
                        nc.vector.tensor_tensor(
                            mulx[DH:128, :], att1[0:DH, :], rb_sb[DH:128, :], MUL
                        )
                        nc.vector.tensor_scalar(
                            att_r[:, pr, sq_sl],
                            mulx[:],
                            bv_t[:, pr : pr + 1],
                            None,
                            ADD,
                        )

            # ================= Phase D: out projection =================
            with (
                tc.tile_pool(name="osb", bufs=3) as osb,
                tc.tile_pool(name="psD", bufs=4, space="PSUM") as psD,
            ):
                for jt in range(NJT):
                    o_ps = [psD.tile([128, 512], F32, tag="psD", name=f"o_ps{jt}_{i}") for i in range(NSQH)]
                    j_sl = slice(128 * jt, 128 * (jt + 1))
                    for pr in range(NPAIR):
                        for sh in range(NSQH):
                            nc.tensor.matmul(
                                o_ps[sh][:],
                                wot_r[:, pr, j_sl],
                                att_r[:, pr, 512 * sh : 512 * (sh + 1)],
                                start=(pr == 0),
                                stop=(pr == NPAIR - 1),
                            )
                    o_sb = osb.tile([128, SQ], F32, tag="osb")
                    for sh in range(NSQH):
                        nc.vector.tensor_copy(
                            o_sb[:, 512 * sh : 512 * (sh + 1)], o_ps[sh][:]
                        )
                    nc.sync.dma_start(out=outp[j_sl, :], in_=o_sb[:])

    _NC_CACHE["nc"] = nc
    return nc


def _make_in_maps(inputs):
    ff = np.asarray(inputs["fused_features"], dtype=np.float32)
    kv_in = np.concatenate(
        [
            np.asarray(inputs["text"], dtype=np.float32),
            np.asarray(inputs["image"], dtype=np.float32),
            np.asarray(inputs["audio"], dtype=np.float32),
            np.asarray(inputs["video"], dtype=np.float32),
        ],
        axis=1,
    )
    Wq = np.asarray(inputs["Wq"], dtype=np.float32)
    Wk = np.asarray(inputs["Wk"], dtype=np.float32)
    Wv = np.asarray(inputs["Wv"], dtype=np.float32)
    Wo = np.asarray(inputs["Wo"], dtype=np.float32)
    bq = np.asarray(inputs["bq"], dtype=np.float32)
    bk = np.asarray(inputs["bk"], dtype=np.float32)
    bv = np.asarray(inputs["bv"], dtype=np.float32)

    import ml_dtypes

    np_mm = np.dtype(ml_dtypes.bfloat16) if NP_MM == "bfloat16" else np.float32
    ffT = [np.ascontiguousarray(ff[b].T.astype(np_mm)) for b in range(B)]
    kvT = [np.ascontiguousarray(kv_in[b].T.astype(np_mm)) for b in range(B)]
    WqT = np.ascontiguousarray(Wq.T.astype(np_mm))  # [4096, 1024]
    WkT = np.ascontiguousarray(Wk.T.astype(np_mm))  # [1024, 1024]
    WvT = np.ascontiguousarray(Wv.T.astype(np_mm))
    WoT = np.ascontiguousarray(Wo.T.astype(np_mm))  # [1024, 4096]

    in_maps = []
    for c in range(NCORES):
        b, hg = divmod(c, HG)
        fs = slice(GF * hg, GF * (hg + 1))
        in_maps.append(
            {
                "fft": ffT[b],
                "kvt": kvT[b],
                "wqt": np.ascontiguousarray(WqT[:, fs]),
                "wkt": np.ascontiguousarray(WkT[:, fs]),
                "wvt": np.ascontiguousarray(WvT[:, fs]),
                "wot": np.ascontiguousarray(WoT[fs, :]),
                "bq": np.ascontiguousarray(bq[fs].reshape(NPAIR, 128).T),
                "bk": np.ascontiguousarray(bk[fs].reshape(NPAIR, 128).T),
                "bv": np.ascontiguousarray(bv[fs].reshape(NPAIR, 128).T),
            }
        )
    return in_maps


def _assemble(results, bo):
    out = np.zeros((B, SQ, DOUT), dtype=np.float32)
    for c in range(NCORES):
        b = c // HG
        out[b] += results[c]["outp"].T
    out += np.asarray(bo, dtype=np.float32)
    return out


def run_spmd(inputs, trace=False):
    nc = build()
    in_maps = _make_in_maps(inputs)
    r = run_bass_kernel_spmd(nc, in_maps, list(range(NCORES)), trace=trace)
    return _assemble(r.results, inputs["bo"]), r


def kernel(**inputs) -> np.ndarray:
    out, _ = run_spmd(inputs, trace=False)
    return out


# revision 8
# speedup vs baseline: 1.4524x; 1.0051x over previous
"""Multi-head cross-attention TRN2 Bass kernel, sharded over 8 NeuronCores.

Problem (nn_MultiHeadCrossAttention): B=2, Sq=1024, Skv=4096 (text+image+
audio+video), hidden=1024, heads=16, head_dim=64, out=4096.

Sharding: core c = 4*b + hg handles batch b and head-group hg (4 heads).
Per core (all matmuls in float32r: ~bf16 speed, ~1e-4 accuracy):
  QT proj:  QT[d,sq]   = Wq_g  @ ff[b].T      (ffT streamed, contraction 4096)
  KT proj:  KT[d,kv]   = Wk_g  @ kv[b].T      (kvT streamed, contraction 1024)
  V  proj:  V[kv,d]    = kv[b] @ Wv_g.T       (natural layout, 65th col = ones)
  scores^T: S[kv,sq]   = K^T q  (row-tiled K=64 matmul pairs)
  softmax:  P = exp(S/8) (no max-subtract: |scores| <~ 3 for this data)
  PV:       att[d,sq] += V_ext^T @ P  (M=65: row 64 accumulates denominator)
  norm:     att = att * recip(den) (K=1 broadcast matmul expands recip row)
  out-proj: outT[j,sq] = Wo[:, fslice].T.T @ attT  -> partial over f-slice
Host sums the 4 per-batch partials and adds bo.
"""

import numpy as np

import bass_rust
import concourse.bass as bass
import concourse.mybir as mybir
import concourse.tile as tile
from concourse.bass_utils import run_bass_kernel_spmd
from concourse.vector_clock import ScopedClock

# ---------------------------------------------------------------------------
# Workarounds for walrus per-instruction sync-wait caps (this walrus build
# rejects instructions carrying more waits than the ISA slot count; Tile's
# sem assignment can attach more). Split excess waits onto single-wait nops.
# ---------------------------------------------------------------------------
import re as _re

_VC_RE = _re.compile(r"VectorClock\(\[([0-9, ]*)\]\)")


def _vc_values(vc):
    m = _VC_RE.match(repr(vc))
    assert m, repr(vc)
    s = m.group(1).strip()
    return [int(x) for x in s.split(",")] if s else []


def _split_excess_waits(tc, ordered_instructions_by_block, max_waits=1):
    nc = tc.nc
    for _bb, insts in ordered_instructions_by_block.items():
        out = []
        for inst in insts:
            si = inst.sync_info
            waits = list(si.on_wait) if si and si.on_wait else []
            if len(waits) > max_waits:
                keep = waits[:max_waits]
                for w in waits[max_waits:]:
                    nop = mybir.InstNoOp(
                        name=nc.get_next_instruction_name(), ins=[], outs=[]
                    )
                    nop.engine = inst.engine
                    nop.sync_info = bass_rust.SyncInfo(on_wait=[w], on_update=[])
                    nc.register_instruction(nop)
                    out.append(nop)
                inst.sync_info = bass_rust.SyncInfo(
                    on_wait=keep, on_update=list(si.on_update or [])
                )
            out.append(inst)
        insts[:] = out


_orig_lower = tile.TileContext._lower_ordered_insts


def _lower_with_split(self, postordered_blocks):
    _split_excess_waits(self, postordered_blocks)
    return _orig_lower(self, postordered_blocks)


def _drain_and_barrier_split(self, tick_clock, wait_clock):
    vals = _vc_values(tick_clock.global_clock)
    for proc_idx, tick in enumerate(vals):
        if tick <= 0:
            continue
        single = [0] * len(vals)
        single[proc_idx] = tick
        nop_inst = self.nc.sync.nop(nofuse=True, hint=f"drain_wait_p{proc_idx}")
        wait_clock.add_sem_waits(
            nop_inst.ins, ScopedClock({None: bass_rust.VectorClock(single)})
        )
    self.nc.sync.drain()
    self.nc.all_engine_barrier()
    assert self.sems is not None
    popped = self.nc._tile_sem_poison_stack.pop()
    assert popped is self._sem_poison
    self.nc.clear_and_free_semaphores(list(self.sems.allocated().values()))
    self.nc.all_engine_barrier()


tile.TileContext._lower_ordered_insts = _lower_with_split
tile.TileContext._drain_and_barrier = _drain_and_barrier_split

# ---------------------------------------------------------------------------
# Problem constants (hardcoded per contract)
# ---------------------------------------------------------------------------
B = 2
SQ = 1024
SKV = 4096
HID = 1024
HEADS = 16
DH = 64
DOUT = 4096
NCORES = 8
HG = 4  # head-groups (cores per batch)
GHEADS = HEADS // HG  # heads per group = 4
GF = GHEADS * DH  # feature slice per group = 256
NPAIR = GHEADS // 2  # head pairs per group = 2

F32 = mybir.dt.float32
F32R = mybir.dt.float32r
BF16 = mybir.dt.bfloat16
FP16 = mybir.dt.float16
DT_MM = BF16  # matmul operand dtype: BF16 (fast ldweights) or F32R (accuracy)
NP_MM = "bfloat16"  # host-side dtype name for DT_MM inputs
Exp = mybir.ActivationFunctionType.Exp
MUL = mybir.AluOpType.mult
ADD = mybir.AluOpType.add

NKVT = SKV // 128  # 32 kv tiles
NKVB = 8  # kv blocks (512 wide)
NFT_Q = 4096 // 128  # 32 contraction tiles for Q proj
NFT_KV = HID // 128  # 8 contraction tiles for K/V proj
NSQH = SQ // 512  # 2 sq halves
NJT = DOUT // 128  # 32 output row tiles

_NC_CACHE = {}


def build():
    if "nc" in _NC_CACHE:
        return _NC_CACHE["nc"]
    nc = bass.Bass()

    fft = nc.declare_dram_parameter("fft", [4096, SQ], DT_MM, isOutput=False)
    kvt = nc.declare_dram_parameter("kvt", [HID, SKV], DT_MM, isOutput=False)
    wqt = nc.declare_dram_parameter("wqt", [4096, GF], DT_MM, isOutput=False)
    wkt = nc.declare_dram_parameter("wkt", [HID, GF], DT_MM, isOutput=False)
    wvt = nc.declare_dram_parameter("wvt", [HID, GF], DT_MM, isOutput=False)
    wot = nc.declare_dram_parameter("wot", [GF, DOUT], DT_MM, isOutput=False)
    bq = nc.declare_dram_parameter("bq", [128, NPAIR], F32, isOutput=False)
    bk = nc.declare_dram_parameter("bk", [128, NPAIR], F32, isOutput=False)
    bv = nc.declare_dram_parameter("bv", [128, NPAIR], F32, isOutput=False)
    outp = nc.declare_dram_parameter("outp", [DOUT, SQ], FP16, isOutput=True)

    with tile.TileContext(nc) as tc:
        with (
            tc.tile_pool(name="hold", bufs=1) as hold,
            tc.tile_pool(name="misc", bufs=1) as misc,
        ):
            # ---- long-lived tiles ----
            wkt_r = hold.tile([128, NFT_KV, NPAIR, 128], DT_MM, tag="wkt")
            nc.sync.dma_start(
                out=wkt_r[:],
                in_=wkt.rearrange("(ft p) (pr d) -> p ft pr d", p=128, pr=NPAIR),
            )
            wvt_r = hold.tile([128, NFT_KV, GF], DT_MM, tag="wvt")
            nc.sync.dma_start(
                out=wvt_r[:], in_=wvt.rearrange("(ft p) d -> p ft d", p=128)
            )
            wot_r = hold.tile([128, NPAIR, DOUT], DT_MM, tag="wot")
            nc.sync.dma_start(
                out=wot_r[:], in_=wot.rearrange("(pr p) j -> p pr j", p=128)
            )
            bq_t = misc.tile([128, NPAIR], F32, tag="bq")
            nc.sync.dma_start(out=bq_t[:], in_=bq[:])
            bk_t = misc.tile([128, NPAIR], F32, tag="bk")
            nc.sync.dma_start(out=bk_t[:], in_=bk[:])
            bv_t = misc.tile([128, NPAIR], F32, tag="bv")
            nc.sync.dma_start(out=bv_t[:], in_=bv[:])

            ones_f = misc.tile([128, GHEADS], F32, tag="ones_f")
            nc.vector.memset(ones_f[:], 1.0)
            ones_row = misc.tile([1, DH], DT_MM, tag="ones_row")
            nc.vector.tensor_copy(ones_row[:], ones_f[0:1, 0:1].broadcast_to([1, DH]))

            qt_r = hold.tile([128, NPAIR, SQ], DT_MM, tag="qt")
            kt_r = hold.tile([128, NPAIR, SKV], DT_MM, tag="kt")
            v_r = hold.tile([128, NKVT, GHEADS, DH + 1], DT_MM, tag="v")
            att_r = hold.tile([128, NPAIR, SQ], DT_MM, tag="att")

            # ================= Phase A: QT projection =================
            with (
                nc.named_scope("phaseA_qt"),
                tc.tile_pool(name="ffts", bufs=6) as ffts,
                tc.tile_pool(name="wqs", bufs=4) as wqs,
                tc.tile_pool(name="psA", bufs=4, space="PSUM") as psA,
            ):
                qt_ps = [
                    psA.tile([128, 512], F32, tag="psA", name=f"qt_ps{i}")
                    for i in range(4)
                ]  # (pair, sqh)
                for kt in range(NFT_Q):
                    fft_t = ffts.tile([128, SQ], DT_MM, tag="fft")
                    nc.sync.dma_start(
                        out=fft_t[:], in_=fft[128 * kt : 128 * (kt + 1), :]
                    )
                    wq_t = wqs.tile([128, NPAIR, 128], DT_MM, tag="wq")
                    nc.sync.dma_start(
                        out=wq_t[:],
                        in_=wqt[128 * kt : 128 * (kt + 1), :].rearrange(
                            "p (pr d) -> p pr d", pr=NPAIR
                        ),
                    )
                    for pr in range(NPAIR):
                        for sh in range(NSQH):
                            nc.tensor.matmul(
                                qt_ps[pr * NSQH + sh][:],
                                wq_t[:, pr, :],
                                fft_t[:, 512 * sh : 512 * (sh + 1)],
                                start=(kt == 0),
                                stop=(kt == NFT_Q - 1),
                            )
                for pr in range(NPAIR):
                    for sh in range(NSQH):
                        nc.vector.tensor_scalar(
                            qt_r[:, pr, 512 * sh : 512 * (sh + 1)],
                            qt_ps[pr * NSQH + sh][:],
                            bq_t[:, pr : pr + 1],
                            None,
                            ADD,
                        )

            # ============ Phase B: KT + V projections (kv blocks) ============
            with (
                nc.named_scope("phaseB_kv"),
                tc.tile_pool(name="kvs", bufs=2) as kvs,
                tc.tile_pool(name="psB", bufs=4, space="PSUM") as psB,
            ):
                for kb in range(NKVB):
                    kv_t = kvs.tile([128, NFT_KV, 512], DT_MM, tag="kv")
                    nc.sync.dma_start(
                        out=kv_t[:],
                        in_=kvt[:, 512 * kb : 512 * (kb + 1)].rearrange(
                            "(ft p) n -> p ft n", p=128
                        ),
                    )
                    for pr in range(NPAIR):
                        kt_ps = psB.tile([128, 512], F32, tag="psB")
                        for ft in range(NFT_KV):
                            nc.tensor.matmul(
                                kt_ps[:],
                                wkt_r[:, ft, pr, :],
                                kv_t[:, ft, :],
                                start=(ft == 0),
                                stop=(ft == NFT_KV - 1),
                            )
                        nc.vector.tensor_scalar(
                            kt_r[:, pr, 512 * kb : 512 * (kb + 1)],
                            kt_ps[:],
                            bk_t[:, pr : pr + 1],
                            None,
                            ADD,
                        )
                    for kl in range(4):
                        kvt_i = kb * 4 + kl
                        v_ps = psB.tile([128, GF], F32, tag="psB")
                        for ft in range(NFT_KV):
                            nc.tensor.matmul(
                                v_ps[:],
                                kv_t[:, ft, 128 * kl : 128 * (kl + 1)],
                                wvt_r[:, ft, :],
                                start=(ft == 0),
                                stop=(ft == NFT_KV - 1),
                            )
                        nc.vector.tensor_copy(
                            v_r[:, kvt_i, :, 0:DH],
                            v_ps.rearrange("p (h d) -> p h d", h=GHEADS),
                        )
                        nc.vector.tensor_copy(
                            v_r[:, kvt_i, :, DH : DH + 1], ones_f[:, :]
                        )

            # ================= Phase C: attention =================
            with (
                nc.named_scope("phaseC_attn"),
                tc.tile_pool(name="pp", bufs=3) as pp,
                tc.tile_pool(name="nrm", bufs=2) as nrm,
                tc.tile_pool(name="psS", bufs=4, space="PSUM") as psS,
                tc.tile_pool(name="psAtt", bufs=2, space="PSUM") as psAtt,
                tc.tile_pool(name="psRb", bufs=2, space="PSUM") as psRb,
            ):
                for pr in range(NPAIR):
                    for sh in range(NSQH):
                        sq_sl = slice(512 * sh, 512 * (sh + 1))
                        att0 = psAtt.tile([DH + 1, 512], F32, tag="psAtt")
                        att1 = psAtt.tile([DH + 1, 512], F32, tag="psAtt")
                        # software pipeline: scores/exp for kv run one tile
                        # ahead of the PV accumulation so the exp latency
                        # never stalls the PE stream.
                        pq = []  # pending (p0, p1) tiles awaiting PV
                        for kv in range(NKVT):
                            s0 = psS.tile([128, 512], F32, tag="psS")
                            s1 = psS.tile([128, 512], F32, tag="psS")
                            kv_sl = slice(128 * kv, 128 * (kv + 1))
                            nc.tensor.matmul(
                                s0[:],
                                kt_r[0:DH, pr, kv_sl],
                                qt_r[0:DH, pr, sq_sl],
                                start=True,
                                stop=True,
                            )
                            nc.tensor.matmul(
                                s1[:],
                                kt_r[DH:128, pr, kv_sl],
                                qt_r[DH:128, pr, sq_sl],
                                start=True,
                                stop=True,
                            )
                            p0 = pp.tile([128, 512], DT_MM, tag="p0")
                            p1 = pp.tile([128, 512], DT_MM, tag="p1")
                            nc.scalar.activation(p0[:], s0[:], Exp, scale=0.125)
                            nc.scalar.activation(p1[:], s1[:], Exp, scale=0.125)
                            pq.append((kv, p0, p1))
                            if kv >= 1:
                                pkv, q0, q1 = pq.pop(0)
                                nc.tensor.matmul(
                                    att0[:],
                                    v_r[:, pkv, 2 * pr, :],
                                    q0[:],
                                    start=(pkv == 0),
                                    stop=False,
                                )
                                nc.tensor.matmul(
                                    att1[:],
                                    v_r[:, pkv, 2 * pr + 1, :],
                                    q1[:],
                                    start=(pkv == 0),
                                    stop=False,
                                )
                        pkv, q0, q1 = pq.pop(0)
                        nc.tensor.matmul(
                            att0[:], v_r[:, pkv, 2 * pr, :], q0[:],
                            start=False, stop=True,
                        )
                        nc.tensor.matmul(
                            att1[:], v_r[:, pkv, 2 * pr + 1, :], q1[:],
                            start=False, stop=True,
                        )
                        # normalize pair
                        rec0 = nrm.tile([1, 512], DT_MM, tag="rec0")
                        rec1 = nrm.tile([1, 512], DT_MM, tag="rec1")
                        with nc.allow_low_precision(reason="softmax recip"):
                            nc.vector.reciprocal(rec0[:], att0[DH : DH + 1, :])
                            nc.vector.reciprocal(rec1[:], att1[DH : DH + 1, :])
                        rb0 = psRb.tile([DH, 512], F32, tag="psRb")
                        rb1 = psRb.tile([DH, 512], F32, tag="psRb")
                        nc.tensor.matmul(
                            rb0[:], ones_row[0:1, :], rec0[0:1, :],
                            start=True, stop=True,
                        )
                        nc.tensor.matmul(
                            rb1[:], ones_row[0:1, :], rec1[0:1, :],
                            start=True, stop=True,
                        )
                        rb_sb = nrm.tile([128, 512], F32, tag="rbsb")
                        nc.vector.tensor_copy(rb_sb[0:DH, :], rb0[:])
                        nc.vector.tensor_copy(rb_sb[DH:128, :], rb1[:])
                        mulx = nrm.tile([128, 512], F32, tag="mulx")
                        nc.vector.tensor_tensor(
                            mulx[0:DH, :], att0[0:DH, :], rb_sb[0:DH, :], MUL
                        )
                        nc.vector.tensor_tensor(
                            mulx[DH:128, :], att1[0:DH, :], rb_sb[DH:128, :], MUL
                        )
                        nc.vector.tensor_scalar(
                            att_r[:, pr, sq_sl],
                            mulx[:],
                            bv_t[:, pr : pr + 1],
                            None,
                            ADD,
                        )

            # ================= Phase D: out projection =================
            with (
                nc.named_scope("phaseD_out"),
                tc.tile_pool(name="osb", bufs=3) as osb,
                tc.tile_pool(name="psD", bufs=4, space="PSUM") as psD,
            ):
                for jt in range(NJT):
                    o_ps = [psD.tile([128, 512], F32, tag="psD", name=f"o_ps{jt}_{i}") for i in range(NSQH)]
                    j_sl = slice(128 * jt, 128 * (jt + 1))
                    for pr in range(NPAIR):
                        for sh in range(NSQH):
                            nc.tensor.matmul(
                                o_ps[sh][:],
                                wot_r[:, pr, j_sl],
                                att_r[:, pr, 512 * sh : 512 * (sh + 1)],
                                start=(pr == 0),
                                stop=(pr == NPAIR - 1),
                            )
                    o_sb = osb.tile([128, SQ], FP16, tag="osb")
                    for sh in range(NSQH):
                        nc.vector.tensor_copy(
                            o_sb[:, 512 * sh : 512 * (sh + 1)], o_ps[sh][:]
                        )
                    nc.sync.dma_start(out=outp[j_sl, :], in_=o_sb[:])

    _NC_CACHE["nc"] = nc
    return nc


def _make_in_maps(inputs):
    ff = np.asarray(inputs["fused_features"], dtype=np.float32)
    kv_in = np.concatenate(
        [
            np.asarray(inputs["text"], dtype=np.float32),
            np.asarray(inputs["image"], dtype=np.float32),
            np.asarray(inputs["audio"], dtype=np.float32),
            np.asarray(inputs["video"], dtype=np.float32),
        ],
        axis=1,
    )
    Wq = np.asarray(inputs["Wq"], dtype=np.float32)
    Wk = np.asarray(inputs["Wk"], dtype=np.float32)
    Wv = np.asarray(inputs["Wv"], dtype=np.float32)
    Wo = np.asarray(inputs["Wo"], dtype=np.float32)
    bq = np.asarray(inputs["bq"], dtype=np.float32)
    bk = np.asarray(inputs["bk"], dtype=np.float32)
    bv = np.asarray(inputs["bv"], dtype=np.float32)

    import ml_dtypes

    np_mm = np.dtype(ml_dtypes.bfloat16) if NP_MM == "bfloat16" else np.float32
    ffT = [np.ascontiguousarray(ff[b].T.astype(np_mm)) for b in range(B)]
    kvT = [np.ascontiguousarray(kv_in[b].T.astype(np_mm)) for b in range(B)]
    WqT = np.ascontiguousarray(Wq.T.astype(np_mm))  # [4096, 1024]
    WkT = np.ascontiguousarray(Wk.T.astype(np_mm))  # [1024, 1024]
    WvT = np.ascontiguousarray(Wv.T.astype(np_mm))
    WoT = np.ascontiguousarray(Wo.T.astype(np_mm))  # [1024, 4096]

    in_maps = []
    for c in range(NCORES):
        b, hg = divmod(c, HG)
        fs = slice(GF * hg, GF * (hg + 1))
        in_maps.append(
            {
                "fft": ffT[b],
                "kvt": kvT[b],
                "wqt": np.ascontiguousarray(WqT[:, fs]),
                "wkt": np.ascontiguousarray(WkT[:, fs]),
                "wvt": np.ascontiguousarray(WvT[:, fs]),
                "wot": np.ascontiguousarray(WoT[fs, :]),
                "bq": np.ascontiguousarray(bq[fs].reshape(NPAIR, 128).T),
                "bk": np.ascontiguousarray(bk[fs].reshape(NPAIR, 128).T),
                "bv": np.ascontiguousarray(bv[fs].reshape(NPAIR, 128).T),
            }
        )
    return in_maps


def _assemble(results, bo):
    out = np.zeros((B, SQ, DOUT), dtype=np.float32)
    for c in range(NCORES):
        b = c // HG
        out[b] += results[c]["outp"].T.astype(np.float32)
    out += np.asarray(bo, dtype=np.float32)
    return out


def run_spmd(inputs, trace=False):
    nc = build()
    in_maps = _make_in_maps(inputs)
    r = run_bass_kernel_spmd(nc, in_maps, list(range(NCORES)), trace=trace)
    return _assemble(r.results, inputs["bo"]), r


def kernel(**inputs) -> np.ndarray:
    out, _ = run_spmd(inputs, trace=False)
    return out
